# revision 8
# baseline (speedup 1.0000x reference)
"""Chamfer loss kernel for Trainium2 (8 NeuronCores, Bass/Tile).

Problem: pred_points [4, 8192, 3] f32, gt_points [4, 8192, 3] f32 ->
scalar mean(min_j d_ij) + mean(min_i d_ij) over squared pairwise dists.

Sharding: core c handles batch c//2, pred rows (c%2)*4096..+4096 against
the full 8192 gt points of that batch (d slab [4096, 8192] per core).

Per-core dataflow (32 row blocks x 4 column groups of [128, 2048] PSUM):
- TensorE: augmented K=13 fp16 matmuls (hi/lo split, ~exact) fill PSUM.
- PSUM tiles admit one serialized reader each, so every group is read
  exactly once: the Scalar engine copies all four groups PSUM->SBUF f16
  into a block-wide df [128, 8192].
- rowmin: DVE pairwise-min fold tree 8192->256 (fp16 2x mode) + min
  reduce -> rowacc[:, i]. Runs one block behind the copies, so the DVE
  queue never blocks the Scalar/PE pipeline.
- colmin: only PAIR-level partials on-chip: pc = min(df_even, df_odd)
  per block pair (fp16 2x), DMA'd out as 16 slices of a [128, 16*8192]
  f16 output. The host finishes the min over pairs/partitions/cores.
Host: cross-core/partition min + mean in numpy (exact, ~0.3s).
"""

import numpy as np

B, N, M, D = 4, 8192, 8192, 3
NCORES = 8
P = 128            # pred rows per block (partition dim)
KAUG = 13          # augmented contraction size
NP = B * N // NCORES   # pred rows per core = 4096
GCOLS = 2048       # gt columns per PSUM group (4 banks)
NMM = 512          # matmul free dim
NPAIR = 16         # block pairs per core (colmin partials shipped)


def build_nc():
    import concourse.bacc as bacc
    import concourse.mybir as mybir
    import concourse.tile as tile

    f16, f32 = mybir.dt.float16, mybir.dt.float32
    MIN = mybir.AluOpType.min

    nblk = NP // P
    ngrp = M // GCOLS
    nmm = GCOLS // NMM

    nc = bacc.Bacc(target_bir_lowering=False)
    lhs = nc.dram_tensor("lhs_aug", [KAUG, NP], f16, kind="ExternalInput")
    rhs = nc.dram_tensor("rhs_aug", [KAUG, M], f16, kind="ExternalInput")
    colpc_o = nc.dram_tensor("colpc", [P, NPAIR * M], f16,
                             kind="ExternalOutput")
    rowmin_o = nc.dram_tensor("rowmin", [P, nblk], f32, kind="ExternalOutput")

    with tile.TileContext(nc) as tc:
        with (
            tc.tile_pool(name="singles", bufs=1) as singles,
            tc.tile_pool(name="scr", bufs=2) as spool,
            tc.tile_pool(name="df", bufs=4) as dfp,
            tc.tile_pool(name="pc", bufs=2) as pcp,
            tc.tile_pool(name="psum", bufs=2, space="PSUM") as ppool,
        ):
            xw = singles.tile([KAUG, NP], f16)
            yw = singles.tile([KAUG, M], f16)
            nc.sync.dma_start(out=xw[:, :], in_=lhs[:, :])
            nc.sync.dma_start(out=yw[:, :], in_=rhs[:, :])

            rowacc = singles.tile([P, nblk], f32)

            def fold_rowmin(buf, i):
                """Pairwise-min fold buf [P, M] down to [P, 256], then
                min-reduce into rowacc[:, i]."""
                prev, w = buf, M // 2
                while w >= 256:
                    f = spool.tile([P, w], f16, tag=f"fold{w}")
                    nc.vector.tensor_tensor(
                        out=f[:, :], in0=prev[:, :w], in1=prev[:, w:2 * w],
                        op=MIN)
                    prev = f
                    w //= 2
                nc.vector.tensor_reduce(
                    out=rowacc[:, i:i + 1], in_=prev[:, :],
                    axis=mybir.AxisListType.X, op=MIN)

            prev_df = None
            for i in range(nblk):
                lhsT = xw[:, i * P:(i + 1) * P]
                df = dfp.tile([P, M], f16, tag="df")
                for g in range(ngrp):
                    cs = slice(g * GCOLS, (g + 1) * GCOLS)
                    ps = ppool.tile([P, GCOLS], f32, tag="ps")
                    for k in range(nmm):
                        nc.tensor.matmul(
                            ps[:, k * NMM:(k + 1) * NMM],
                            lhsT,
                            yw[:, g * GCOLS + k * NMM:g * GCOLS + (k + 1) * NMM],
                            start=True,
                            stop=True,
                        )
                    nc.scalar.copy(df[:, cs], ps[:, :])
                fold_rowmin(df, i)
                if i % 2 == 0:
                    prev_df = df
                else:
                    pi = i // 2
                    pc = pcp.tile([P, M], f16, tag="pc")
                    nc.vector.tensor_tensor(
                        out=pc[:, :], in0=prev_df[:, :], in1=df[:, :], op=MIN)
                    nc.sync.dma_start(
                        out=colpc_o[:, pi * M:(pi + 1) * M], in_=pc[:, :])
                    prev_df = None
            nc.sync.dma_start(out=rowmin_o[:, :], in_=rowacc[:, :])
    nc.finalize()
    return nc


def _augment(x, y):
    """x [n,3] f32 pred block, y [m,3] f32 gt -> (lhs_aug [13,n] f16,
    rhs_aug [13,m] f16) such that lhs.T @ rhs ~= squared distance matrix."""
    f16, f32 = np.float16, np.float32
    x = np.ascontiguousarray(x, dtype=f32)
    y = np.ascontiguousarray(y, dtype=f32)
    x2 = (x * x).sum(-1)
    y2 = (y * y).sum(-1)
    xh = x.astype(f16)
    xl = (x - xh.astype(f32)).astype(f16)
    yh = y.astype(f16)
    yl = (y - yh.astype(f32)).astype(f16)
    x2h = x2.astype(f16)
    x2l = (x2 - x2h.astype(f32)).astype(f16)
    y2h = y2.astype(f16)
    y2l = (y2 - y2h.astype(f32)).astype(f16)
    m2yh = (yh.astype(f32) * -2.0).astype(f16)   # exact: x2 scaling
    m2yl = (yl.astype(f32) * -2.0).astype(f16)
    n, m = x.shape[0], y.shape[0]
    ones_n = np.ones(n, f16)
    ones_m = np.ones(m, f16)
    lhs = np.stack([
        xh[:, 0], xh[:, 1], xh[:, 2],
        xh[:, 0], xh[:, 1], xh[:, 2],
        xl[:, 0], xl[:, 1], xl[:, 2],
        x2h, x2l, ones_n, ones_n,
    ])  # [13, n]
    rhsa = np.stack([
        m2yh[:, 0], m2yh[:, 1], m2yh[:, 2],
        m2yl[:, 0], m2yl[:, 1], m2yl[:, 2],
        m2yh[:, 0], m2yh[:, 1], m2yh[:, 2],
        ones_m, ones_m, y2h, y2l,
    ])  # [13, m]
    return np.ascontiguousarray(lhs), np.ascontiguousarray(rhsa)


def _make_in_maps(pred_points, gt_points):
    pred = np.asarray(pred_points, dtype=np.float32)
    gt = np.asarray(gt_points, dtype=np.float32)
    in_maps = []
    for c in range(NCORES):
        b, h = c // 2, c % 2
        lhs, rhsa = _augment(pred[b, h * NP:(h + 1) * NP], gt[b])
        in_maps.append({"lhs_aug": lhs, "rhs_aug": rhsa})
    return in_maps


def _finish(results):
    """results: list per core of {'rowmin': [128, nblk] f32,
    'colmin': [128, M] f16} -> scalar chamfer loss."""
    rowsum = np.float64(0.0)
    colsum = np.float64(0.0)
    for c in range(NCORES):
        r = np.maximum(results[c]["rowmin"].astype(np.float64), 0.0)
        rowsum += r.sum()
    for b in range(B):
        m = np.minimum(
            results[2 * b]["colpc"].reshape(P, NPAIR, M)
            .astype(np.float32).min(axis=(0, 1)),
            results[2 * b + 1]["colpc"].reshape(P, NPAIR, M)
            .astype(np.float32).min(axis=(0, 1)),
        )
        cm = np.maximum(m.astype(np.float64), 0.0)
        colsum += cm.sum()
    total = rowsum / (B * N) + colsum / (B * M)
    return np.float32(total)


_RUN_CACHE = {}


def _run_on_hw(in_maps, trace=False, **kw):
    from concourse.bass_utils import run_bass_kernel_spmd

    nc = _RUN_CACHE.get("nc")
    if nc is None:
        nc = build_nc()
        _RUN_CACHE["nc"] = nc
    return run_bass_kernel_spmd(
        nc, in_maps, core_ids=list(range(NCORES)), trace=trace, **kw
    )


def kernel(pred_points, gt_points):
    in_maps = _make_in_maps(pred_points, gt_points)
    br = _run_on_hw(in_maps, trace=False)
    return _finish(br.results)


if __name__ == "__main__":
    pred = np.random.randn(B, N, D).astype(np.float32)
    gt = np.random.randn(B, M, D).astype(np.float32)
    print(kernel(pred, gt))



# revision 9
# speedup vs baseline: 1.1350x; 1.1350x over previous
"""Chamfer loss kernel for Trainium2 (8 NeuronCores, Bass/Tile).

Problem: pred_points [4, 8192, 3] f32, gt_points [4, 8192, 3] f32 ->
scalar mean(min_j d_ij) + mean(min_i d_ij) over squared pairwise dists.

Sharding: core c handles batch c//2, pred rows (c%2)*4096..+4096 against
the full 8192 gt points of that batch (d slab [4096, 8192] per core).

Per-core dataflow (32 row blocks x 4 column groups of [128, 2048] PSUM):
- TensorE: augmented K=13 fp16 matmuls (hi/lo split, ~exact) fill PSUM.
- PSUM tiles admit one serialized reader each, so every group is read
  exactly once: the Scalar engine copies all four groups PSUM->SBUF f16
  into a block-wide df [128, 8192].
- rowmin: DVE pairwise-min fold tree 8192->256 (fp16 2x mode) + min
  reduce -> rowacc[:, i]. Runs one block behind the copies, so the DVE
  queue never blocks the Scalar/PE pipeline.
- colmin: only PAIR-level partials on-chip: pc = min(df_even, df_odd)
  per block pair (fp16 2x), DMA'd out as 16 slices of a [128, 16*8192]
  f16 output. The host finishes the min over pairs/partitions/cores.
Host: cross-core/partition min + mean in numpy (exact, ~0.3s).
"""

import numpy as np

B, N, M, D = 4, 8192, 8192, 3
NCORES = 8
P = 128            # pred rows per block (partition dim)
KAUG = 13          # augmented contraction size
NP = B * N // NCORES   # pred rows per core = 4096
GCOLS = 2048       # gt columns per PSUM group (4 banks)
NMM = 512          # matmul free dim
NPAIR = 8          # block quads per core (colmin partials shipped)


def build_nc():
    import concourse.bacc as bacc
    import concourse.mybir as mybir
    import concourse.tile as tile

    f16, f32 = mybir.dt.float16, mybir.dt.float32
    MIN = mybir.AluOpType.min

    nblk = NP // P
    ngrp = M // GCOLS
    nmm = GCOLS // NMM

    nc = bacc.Bacc(target_bir_lowering=False)
    lhs = nc.dram_tensor("lhs_aug", [KAUG, NP], f16, kind="ExternalInput")
    rhs = nc.dram_tensor("rhs_aug", [KAUG, M], f16, kind="ExternalInput")
    colpc_o = nc.dram_tensor("colpc", [P, NPAIR * M], f16,
                             kind="ExternalOutput")
    rowmin_o = nc.dram_tensor("rowmin", [P, nblk], f32, kind="ExternalOutput")

    with tile.TileContext(nc) as tc:
        with (
            tc.tile_pool(name="singles", bufs=1) as singles,
            tc.tile_pool(name="scr", bufs=2) as spool,
            tc.tile_pool(name="df", bufs=4) as dfp,
            tc.tile_pool(name="pc", bufs=2) as pcp,
            tc.tile_pool(name="qc", bufs=2) as qcp,
            tc.tile_pool(name="psum", bufs=2, space="PSUM") as ppool,
        ):
            xw = singles.tile([KAUG, NP], f16)
            yw = singles.tile([KAUG, M], f16)
            nc.sync.dma_start(out=xw[:, :], in_=lhs[:, :])
            for c in range(4):
                ys = slice(c * (M // 4), (c + 1) * (M // 4))
                nc.sync.dma_start(out=yw[:, ys], in_=rhs[:, ys])

            rowacc = singles.tile([P, nblk], f32)

            def fold_rowmin(buf, i):
                """Pairwise-min fold buf [P, M] down to [P, 256], then
                min-reduce into rowacc[:, i]."""
                prev, w = buf, M // 2
                while w >= 256:
                    f = spool.tile([P, w], f16, tag=f"fold{w}")
                    nc.vector.tensor_tensor(
                        out=f[:, :], in0=prev[:, :w], in1=prev[:, w:2 * w],
                        op=MIN)
                    prev = f
                    w //= 2
                nc.vector.tensor_reduce(
                    out=rowacc[:, i:i + 1], in_=prev[:, :],
                    axis=mybir.AxisListType.X, op=MIN)

            prev_df = None
            prev_pc = None
            for i in range(nblk):
                lhsT = xw[:, i * P:(i + 1) * P]
                df = dfp.tile([P, M], f16, tag="df")
                for g in range(ngrp):
                    cs = slice(g * GCOLS, (g + 1) * GCOLS)
                    ps = ppool.tile([P, GCOLS], f32, tag="ps")
                    for k in range(nmm):
                        nc.tensor.matmul(
                            ps[:, k * NMM:(k + 1) * NMM],
                            lhsT,
                            yw[:, g * GCOLS + k * NMM:g * GCOLS + (k + 1) * NMM],
                            start=True,
                            stop=True,
                        )
                    nc.scalar.copy(df[:, cs], ps[:, :])
                fold_rowmin(df, i)
                if i % 2 == 0:
                    prev_df = df
                else:
                    pc = pcp.tile([P, M], f16, tag="pc")
                    nc.vector.tensor_tensor(
                        out=pc[:, :], in0=prev_df[:, :], in1=df[:, :], op=MIN)
                    prev_df = None
                    if i % 4 == 1:
                        prev_pc = pc
                    else:
                        qi = i // 4
                        qc = qcp.tile([P, M], f16, tag="qc")
                        nc.vector.tensor_tensor(
                            out=qc[:, :], in0=prev_pc[:, :], in1=pc[:, :],
                            op=MIN)
                        nc.sync.dma_start(
                            out=colpc_o[:, qi * M:(qi + 1) * M], in_=qc[:, :])
                        prev_pc = None
            nc.sync.dma_start(out=rowmin_o[:, :], in_=rowacc[:, :])
    nc.finalize()
    return nc


def _augment(x, y):
    """x [n,3] f32 pred block, y [m,3] f32 gt -> (lhs_aug [13,n] f16,
    rhs_aug [13,m] f16) such that lhs.T @ rhs ~= squared distance matrix."""
    f16, f32 = np.float16, np.float32
    x = np.ascontiguousarray(x, dtype=f32)
    y = np.ascontiguousarray(y, dtype=f32)
    x2 = (x * x).sum(-1)
    y2 = (y * y).sum(-1)
    xh = x.astype(f16)
    xl = (x - xh.astype(f32)).astype(f16)
    yh = y.astype(f16)
    yl = (y - yh.astype(f32)).astype(f16)
    x2h = x2.astype(f16)
    x2l = (x2 - x2h.astype(f32)).astype(f16)
    y2h = y2.astype(f16)
    y2l = (y2 - y2h.astype(f32)).astype(f16)
    m2yh = (yh.astype(f32) * -2.0).astype(f16)   # exact: x2 scaling
    m2yl = (yl.astype(f32) * -2.0).astype(f16)
    n, m = x.shape[0], y.shape[0]
    ones_n = np.ones(n, f16)
    ones_m = np.ones(m, f16)
    lhs = np.stack([
        xh[:, 0], xh[:, 1], xh[:, 2],
        xh[:, 0], xh[:, 1], xh[:, 2],
        xl[:, 0], xl[:, 1], xl[:, 2],
        x2h, x2l, ones_n, ones_n,
    ])  # [13, n]
    rhsa = np.stack([
        m2yh[:, 0], m2yh[:, 1], m2yh[:, 2],
        m2yl[:, 0], m2yl[:, 1], m2yl[:, 2],
        m2yh[:, 0], m2yh[:, 1], m2yh[:, 2],
        ones_m, ones_m, y2h, y2l,
    ])  # [13, m]
    return np.ascontiguousarray(lhs), np.ascontiguousarray(rhsa)


def _make_in_maps(pred_points, gt_points):
    pred = np.asarray(pred_points, dtype=np.float32)
    gt = np.asarray(gt_points, dtype=np.float32)
    in_maps = []
    for c in range(NCORES):
        b, h = c // 2, c % 2
        lhs, rhsa = _augment(pred[b, h * NP:(h + 1) * NP], gt[b])
        in_maps.append({"lhs_aug": lhs, "rhs_aug": rhsa})
    return in_maps


def _finish(results):
    """results: list per core of {'rowmin': [128, nblk] f32,
    'colmin': [128, M] f16} -> scalar chamfer loss."""
    rowsum = np.float64(0.0)
    colsum = np.float64(0.0)
    for c in range(NCORES):
        r = np.maximum(results[c]["rowmin"].astype(np.float64), 0.0)
        rowsum += r.sum()
    for b in range(B):
        m = np.minimum(
            results[2 * b]["colpc"].reshape(P, NPAIR, M)
            .astype(np.float32).min(axis=(0, 1)),
            results[2 * b + 1]["colpc"].reshape(P, NPAIR, M)
            .astype(np.float32).min(axis=(0, 1)),
        )
        cm = np.maximum(m.astype(np.float64), 0.0)
        colsum += cm.sum()
    total = rowsum / (B * N) + colsum / (B * M)
    return np.float32(total)


_RUN_CACHE = {}


def _run_on_hw(in_maps, trace=False, **kw):
    from concourse.bass_utils import run_bass_kernel_spmd

    nc = _RUN_CACHE.get("nc")
    if nc is None:
        nc = build_nc()
        _RUN_CACHE["nc"] = nc
    return run_bass_kernel_spmd(
        nc, in_maps, core_ids=list(range(NCORES)), trace=trace, **kw
    )


def kernel(pred_points, gt_points):
    in_maps = _make_in_maps(pred_points, gt_points)
    br = _run_on_hw(in_maps, trace=False)
    return _finish(br.results)


if __name__ == "__main__":
    pred = np.random.randn(B, N, D).astype(np.float32)
    gt = np.random.randn(B, M, D).astype(np.float32)
    print(kernel(pred, gt))

